# revision 1
# baseline (speedup 1.0000x reference)
"""BitLinear forward (RMSNorm -> int8 activation quant -> ternary weight quant
-> matmul -> rescale) on 8 Trainium2 NeuronCores.

Sharding: data-parallel over rows. x (4,4096,1024) flattens to (16384,1024);
each core gets 2048 rows and the full weight (4096,1024). w_scale=mean|w| is
either computed locally from the full weight (two-pass) or, when collectives
are enabled, from a per-core shard followed by a scalar AllReduce (one-pass).
Output (16384,4096) is concatenated on the host and reshaped.

Math notes:
 - x_q are exact integers in [-128,127] and w_t in {-1,0,1}; both are exact in
   bf16, so a bf16 matmul with fp32 PSUM accumulation reproduces the fp32
   reference einsum bit-for-bit (|sums| < 2^24).
 - round-half-to-even is done in fp32 via the magic constant 1.5*2^23.
 - ternary quantize sign(ws)*(|ws|>0.5) == RNE(clip(ws,-1,1)) exactly.
 - transposes to [k, r]/[k, n] layouts are identity matmuls (out = a.T @ I),
   batched 4 chunks per PSUM bank with one wide copy back to SBUF.
"""

import os

import numpy as np

import concourse.bass as bass
import concourse.mybir as mybir
import concourse.tile as tile
from concourse import bacc
from concourse.bass_utils import run_bass_kernel_spmd
from concourse.masks import make_identity
from concourse import bass_isa

F32 = mybir.dt.float32
BF16 = mybir.dt.bfloat16
ALU = mybir.AluOpType
AF = mybir.ActivationFunctionType

N_CORES = 8
R_FULL, K, N = 16384, 1024, 4096
R = R_FULL // N_CORES          # 2048 rows per core
RT = R // 128                  # 16 row tiles per core
KC = K // 128                  # 8 k-chunks
WS = N // 128                  # 32 weight strips (of 128 out-features)
WSH = WS // N_CORES            # 4 strips per core in the w_scale shard
NH = 2                         # n halves (2048 each)
NQ = 4                         # 512-wide psum tiles per half

C_MAGIC = 12582912.0           # 1.5 * 2^23: fp32 round-to-nearest-even trick
Q_EPS = 1e-5
NORM_EPS = 1e-6


def build_nc(g_is_ones: bool, use_collective: bool):
    nc = bacc.Bacc("TRN2", target_bir_lowering=False)

    x_d = nc.dram_tensor("x", [R, K], F32, kind="ExternalInput")
    w_d = nc.dram_tensor("w", [N, K], F32, kind="ExternalInput")
    if use_collective:
        wsh_d = nc.dram_tensor("wshard", [WSH * 128, K], F32, kind="ExternalInput")
        cc_in = nc.dram_tensor("cc_in", [1, 1], F32)
        cc_out = nc.dram_tensor("cc_out", [1, 1], F32, addr_space="Shared")
    if not g_is_ones:
        g_d = nc.dram_tensor("g", [1, K], F32, kind="ExternalInput")
    out_d = nc.dram_tensor("out", [R, N], F32, kind="ExternalOutput")

    with tile.TileContext(nc) as tc:
        with (
            tc.tile_pool(name="persist", bufs=1) as persist,
            tc.tile_pool(name="wst", bufs=6) as wst_pool,
            tc.tile_pool(name="wshp", bufs=2) as wsh_pool,
            tc.tile_pool(name="wscr", bufs=2) as wscr_pool,
            tc.tile_pool(name="xp", bufs=2) as x_pool,
            tc.tile_pool(name="xqTp", bufs=16) as xqT_pool,
            tc.tile_pool(name="csp", bufs=16) as cs_pool,
            tc.tile_pool(name="big", bufs=2) as big_pool,
            tc.tile_pool(name="stats", bufs=4) as st_pool,
            tc.tile_pool(name="osb", bufs=3) as osb_pool,
            tc.tile_pool(name="pmm", bufs=6, space="PSUM") as psum_mm,
            tc.tile_pool(name="ptp", bufs=2, space="PSUM") as psum_tp,
        ):
            # ---- constants ----
            ident = persist.tile([128, 128], BF16, tag="ident")
            make_identity(nc, ident[:])
            cb = persist.tile([128, 1], F32, tag="cb")
            nc.vector.memset(cb[:], C_MAGIC)

            if not g_is_ones:
                g_row = persist.tile([1, K], F32, tag="g_row")
                nc.sync.dma_start(g_row[:], g_d[:])
                g_b = persist.tile([128, K], F32, tag="g_b")
                nc.gpsimd.partition_broadcast(g_b[:], g_row[0:1, :])

            # w_t^T, bf16, split by n-half so matmuls only depend on the
            # strips of their half: wTT[h][:, j, n] = w_t[h*2048+n, j*128+kk]
            wTT = [
                persist.tile([128, KC, N // NH], BF16, tag=f"wTT{h}", name=f"wTT{h}")
                for h in range(NH)
            ]

            # ---- w_scale: sum of |w| ----
            nsum = WSH if use_collective else WS
            wpart = persist.tile([128, nsum], F32, tag="wpart")
            wsrc = wsh_d if use_collective else w_d
            with nc.named_scope("w_abs_sum"):
                for s in range(nsum):
                    wsha = wsh_pool.tile([128, K], F32, tag="wsha", name=f"wsha{s}")
                    nc.sync.dma_start(wsha[:], wsrc[s * 128:(s + 1) * 128, :])
                    wabs = wscr_pool.tile([128, K], BF16, tag="wabs", name=f"wabs{s}")
                    nc.scalar.activation(
                        wabs[:], wsha[:], AF.Abs,
                        accum_out=wpart[:, s:s + 1],
                    )
                wall = st_pool.tile([128, nsum], F32, tag="wall")
                nc.gpsimd.partition_all_reduce(
                    wall[:], wpart[:], channels=128,
                    reduce_op=bass_isa.ReduceOp.add)
                wsumb = st_pool.tile([128, 1], F32, tag="wsumb")
                nc.vector.reduce_sum(wsumb[:], wall[:], axis=mybir.AxisListType.X)

                if use_collective:
                    nc.sync.dma_start(cc_in[:], wsumb[0:1, :])
                    nc.gpsimd.collective_compute(
                        "AllReduce",
                        ALU.add,
                        replica_groups=[list(range(N_CORES))],
                        ins=[cc_in[:]],
                        outs=[cc_out[:]],
                    )
                    wsum_g1 = st_pool.tile([1, 1], F32, tag="wsum_g1")
                    nc.sync.dma_start(wsum_g1[:], cc_out[:])
                    wsum_g = st_pool.tile([128, 1], F32, tag="wsum_g")
                    nc.gpsimd.partition_broadcast(wsum_g[:], wsum_g1[0:1, :])
                else:
                    wsum_g = wsumb

                # w_scale = mean|w|; inv = 1/(w_scale + eps)  (all [128,1])
                wsb = persist.tile([128, 1], F32, tag="wsb")
                nc.vector.tensor_scalar(
                    out=wsb[:], in0=wsum_g[:], scalar1=1.0 / (N * K),
                    scalar2=None, op0=ALU.mult)
                speps1 = st_pool.tile([128, 1], F32, tag="speps1")
                nc.vector.tensor_scalar(
                    out=speps1[:], in0=wsum_g[:], scalar1=1.0 / (N * K),
                    scalar2=Q_EPS, op0=ALU.mult, op1=ALU.add)
                invb = persist.tile([128, 1], F32, tag="invb")
                nc.vector.reciprocal(invb[:], speps1[:])

            # ---- x quantization (independent of w) ----
            xqT_tiles = []
            cs_tiles = []

            def emit_x(rt):
                with nc.named_scope("x_quant"):
                    xt = x_pool.tile([128, K], F32, tag="xt", name=f"xt{rt}")
                    nc.scalar.dma_start(xt[:], x_d[rt * 128:(rt + 1) * 128, :])

                    if g_is_ones:
                        xg = xt
                    else:
                        xg = big_pool.tile([128, K], F32, tag="xg", name=f"xg{rt}")
                        nc.vector.tensor_mul(xg[:], xt[:], g_b[:])

                    xsq = big_pool.tile([128, K], BF16, tag="xsq", name=f"xsq{rt}")
                    ssq = st_pool.tile([128, 1], F32, tag="ssq")
                    nc.vector.scalar_tensor_tensor(
                        out=xsq[:], in0=xt[:], scalar=1.0, in1=xt[:],
                        op0=ALU.mult, op1=ALU.mult, accum_out=ssq[:])
                    am = st_pool.tile([128, 1], F32, tag="am")
                    nc.vector.tensor_reduce(
                        am[:], xg[:], axis=mybir.AxisListType.X, op=ALU.max,
                        apply_absolute_value=True)

                    # rs = 1/sqrt(ms + eps) with one Newton step on sqrt
                    ms = st_pool.tile([128, 1], F32, tag="ms")
                    nc.vector.tensor_scalar(
                        out=ms[:], in0=ssq[:], scalar1=1.0 / K,
                        scalar2=NORM_EPS, op0=ALU.mult, op1=ALU.add)
                    s0 = st_pool.tile([128, 1], F32, tag="s0")
                    nc.scalar.sqrt(s0[:], ms[:])
                    r0 = st_pool.tile([128, 1], F32, tag="r0")
                    nc.vector.reciprocal(r0[:], s0[:])
                    t0 = st_pool.tile([128, 1], F32, tag="t0")
                    nc.vector.tensor_mul(t0[:], ms[:], r0[:])
                    t1 = st_pool.tile([128, 1], F32, tag="t1")
                    nc.vector.tensor_add(t1[:], t0[:], s0[:])
                    s1 = st_pool.tile([128, 1], F32, tag="s1")
                    nc.vector.tensor_scalar(
                        out=s1[:], in0=t1[:], scalar1=0.5,
                        scalar2=None, op0=ALU.mult)
                    rs = st_pool.tile([128, 1], F32, tag="rs")
                    nc.vector.reciprocal(rs[:], s1[:])

                    axr = st_pool.tile([128, 1], F32, tag="axr")
                    nc.vector.tensor_mul(axr[:], am[:], rs[:])
                    xsc = st_pool.tile([128, 1], F32, tag="xsc")
                    nc.vector.tensor_scalar(
                        out=xsc[:], in0=axr[:], scalar1=1.0 / 127.0,
                        scalar2=None, op0=ALU.mult)
                    cs = cs_pool.tile([128, 1], F32, tag="cs", name=f"cs{rt}")
                    nc.vector.tensor_mul(cs[:], xsc[:], wsb[:])
                    sx = st_pool.tile([128, 1], F32, tag="sx")
                    nc.vector.tensor_scalar(
                        out=sx[:], in0=axr[:], scalar1=1.0 / 127.0,
                        scalar2=Q_EPS, op0=ALU.mult, op1=ALU.add)
                    dx = st_pool.tile([128, 1], F32, tag="dx")
                    nc.vector.reciprocal(dx[:], sx[:])
                    srow = st_pool.tile([128, 1], F32, tag="srow")
                    nc.vector.tensor_mul(srow[:], rs[:], dx[:])

                    # x_q = RNE(xg * srow) via +C (ACT) then -C (DVE, to bf16)
                    ux = big_pool.tile([128, K], F32, tag="ux", name=f"ux{rt}")
                    nc.scalar.activation(
                        ux[:], xg[:], AF.Identity,
                        bias=cb[:, 0:1], scale=srow[:, 0:1])
                    xq = big_pool.tile([128, K], BF16, tag="xq", name=f"xq{rt}")
                    nc.vector.tensor_scalar(
                        out=xq[:], in0=ux[:], scalar1=C_MAGIC,
                        scalar2=None, op0=ALU.subtract)

                    # transpose via identity matmuls, 4 chunks per psum
                    # bank, one wide copy back per bank
                    xqT = xqT_pool.tile([128, KC, 128], BF16, tag="xqT",
                                      name=f"xqT{rt}")
                    for g in range(KC // 4):
                        tpx = psum_tp.tile([128, 512], F32, tag="tp",
                                           name=f"tpx{rt}_{g}")
                        for jj in range(4):
                            j = g * 4 + jj
                            nc.tensor.matmul(
                                tpx[:, jj * 128:(jj + 1) * 128],
                                lhsT=xq[:, j * 128:(j + 1) * 128],
                                rhs=ident[:])
                        if g == 0:
                            nc.vector.tensor_copy(
                                xqT[:, g * 4:(g + 1) * 4, :], tpx[:])
                        else:
                            nc.scalar.copy(
                                xqT[:, g * 4:(g + 1) * 4, :], tpx[:])
                xqT_tiles.append(xqT)
                cs_tiles.append(cs)

            # ---- ternarize + transpose w strip (after thr; single pass when
            #      collective, re-streamed otherwise) ----
            def emit_w2(s):
                with nc.named_scope("w_ternarize"):
                    wst2 = wst_pool.tile([128, K], F32, tag="wst2", name=f"wst2{s}")
                    nc.sync.dma_start(wst2[:], w_d[s * 128:(s + 1) * 128, :])
                    u = wscr_pool.tile([128, K], F32, tag="wu", name=f"wu{s}")
                    nc.vector.tensor_scalar(
                        out=u[:], in0=wst2[:], scalar1=invb[:, 0:1],
                        scalar2=1.0, op0=ALU.mult, op1=ALU.min)
                    v = wscr_pool.tile([128, K], F32, tag="wv", name=f"wv{s}")
                    nc.vector.tensor_scalar(
                        out=v[:], in0=u[:], scalar1=-1.0,
                        scalar2=C_MAGIC, op0=ALU.max, op1=ALU.add)
                    wtn = wscr_pool.tile([128, K], BF16, tag="wtn", name=f"wtn{s}")
                    nc.scalar.activation(wtn[:], v[:], AF.Copy, bias=-C_MAGIC)

                    h, hcol = s // (WS // NH), (s % (WS // NH)) * 128
                    for g in range(KC // 4):
                        tpw = psum_tp.tile([128, 512], F32, tag="tp",
                                           name=f"tpw{s}_{g}")
                        for jj in range(4):
                            j = g * 4 + jj
                            nc.tensor.matmul(
                                tpw[:, jj * 128:(jj + 1) * 128],
                                lhsT=wtn[:, j * 128:(j + 1) * 128],
                                rhs=ident[:])
                        dst = wTT[h][:, g * 4:(g + 1) * 4, hcol:hcol + 128]
                        if g == 0:
                            nc.vector.tensor_copy(dst, tpw[:])
                        else:
                            nc.scalar.copy(dst, tpw[:])

            # ---- matmul + rescale for one (row-tile, n-half) ----
            def emit_mm(rt, h):
                xqT = xqT_tiles[rt]
                cs = cs_tiles[rt]
                with nc.named_scope("mm"):
                    pst = [
                        psum_mm.tile([128, 512], F32, tag="pmm",
                                     name=f"pmm_{rt}_{h}_{q}")
                        for q in range(NQ)
                    ]
                    for j in range(KC):
                        for q in range(NQ):
                            nc.tensor.matmul(
                                pst[q][:],
                                lhsT=xqT[:, j, :],
                                rhs=wTT[h][:, j, q * 512:(q + 1) * 512],
                                start=(j == 0), stop=(j == KC - 1))
                with nc.named_scope("out_scale"):
                    osbh = osb_pool.tile([128, N // NH], F32, tag="osb",
                                         name=f"osb{rt}_{h}")
                    for q in range(NQ):
                        dst = osbh[:, q * 512:(q + 1) * 512]
                        if q < NQ // 2:
                            nc.scalar.activation(
                                dst, pst[q][:], AF.Copy, scale=cs[:, 0:1])
                        else:
                            nc.vector.tensor_scalar(
                                out=dst, in0=pst[q][:], scalar1=cs[:, 0:1],
                                scalar2=None, op0=ALU.mult)
                    eng = nc.sync if (rt + h) % 2 == 0 else nc.scalar
                    eng.dma_start(
                        out_d[rt * 128:(rt + 1) * 128,
                              h * 2048:(h + 1) * 2048],
                        osbh[:])

            # ---- emission schedule ----
            # interleave x tiles with w strips; h0 matmuls join once their
            # dependencies are emitted; then drain h1.
            for i in range(8):
                emit_x(i)
                emit_w2(2 * i)
                emit_w2(2 * i + 1)
            for i in range(8, 16):
                emit_x(i)
                emit_w2(2 * i)
                emit_w2(2 * i + 1)
                emit_mm(i - 8, 0)
            for rt in range(8, 16):
                emit_mm(rt, 0)
                emit_mm(rt - 8, 1)
            for rt in range(8, 16):
                emit_mm(rt, 1)

    nc.compile()
    return nc


def _ensure_ntff_hook():
    """Make trace=True work: bass_utils imports antenv.axon_hooks, which is
    not present in this image. Shim it and install the ctypes-based NTFF
    profiling hook against libaxon_pjrt.so (same recipe as trn_boot)."""
    import sys
    import types
    try:
        import antenv.axon_hooks  # noqa: F401
        return
    except ImportError:
        pass
    mod = types.ModuleType("antenv.axon_hooks")
    mod._hook = None
    mod.set_axon_ntff_profile_hook = lambda h: setattr(mod, "_hook", h)
    mod.get_axon_ntff_profile_hook = lambda: mod._hook
    sys.modules["antenv.axon_hooks"] = mod
    import antenv
    antenv.axon_hooks = mod
    try:
        from trn_agent_boot.trn_boot import _ntff_profile_via_ctypes
        hook = _ntff_profile_via_ctypes("/opt/axon/libaxon_pjrt.so")
        if hook is not None:
            mod._hook = hook
    except Exception as e:  # degrade to no-trace
        print(f"ntff hook install failed: {e}")
    # no S3 in this sandbox; keep artifacts local
    import concourse.bass_utils as bu
    bu.upload_artifacts = lambda tmpdir: f"local://{tmpdir}"


_NC_CACHE = {}


def kernel(x: np.ndarray, weight: np.ndarray, norm_weight: np.ndarray) -> np.ndarray:
    x = np.ascontiguousarray(x, dtype=np.float32)
    weight = np.ascontiguousarray(weight, dtype=np.float32)
    norm_weight = np.ascontiguousarray(norm_weight, dtype=np.float32)

    B, S, Kin = x.shape
    xf = x.reshape(-1, Kin)
    g_is_ones = bool(np.all(norm_weight == 1.0))
    use_collective = bool(int(os.environ.get("BITLIN_COLLECTIVE", "0")))

    key = (g_is_ones, use_collective)
    if key not in _NC_CACHE:
        _NC_CACHE[key] = build_nc(g_is_ones, use_collective)
    nc = _NC_CACHE[key]

    in_maps = []
    for i in range(N_CORES):
        m = {"x": xf[i * R:(i + 1) * R], "w": weight}
        if use_collective:
            m["wshard"] = weight[i * WSH * 128:(i + 1) * WSH * 128]
        if not g_is_ones:
            m["g"] = norm_weight.reshape(1, Kin)
        in_maps.append(m)

    trace = bool(int(os.environ.get("BITLIN_TRACE", "0")))
    if trace:
        _ensure_ntff_hook()
    res = run_bass_kernel_spmd(
        nc, in_maps, core_ids=list(range(N_CORES)), trace=trace,
    )
    if trace:
        kernel.last_results = res
    out = np.concatenate([r["out"] for r in res.results], axis=0)
    return out.reshape(B, S, weight.shape[0]).astype(np.float32)



# revision 2
# speedup vs baseline: 1.2917x; 1.2917x over previous
"""BitLinear forward (RMSNorm -> int8 activation quant -> ternary weight quant
-> matmul -> rescale) on 8 Trainium2 NeuronCores.

Sharding: data-parallel over rows. x (4,4096,1024) flattens to (16384,1024);
each core gets 2048 rows and the full weight (4096,1024). w_scale=mean|w| is
either computed locally from the full weight (two-pass) or, when collectives
are enabled, from a per-core shard followed by a scalar AllReduce (one-pass).
Output (16384,4096) is concatenated on the host and reshaped.

Math notes:
 - x_q are exact integers in [-128,127] and w_t in {-1,0,1}; both are exact in
   bf16, so a bf16 matmul with fp32 PSUM accumulation reproduces the fp32
   reference einsum bit-for-bit (|sums| < 2^24).
 - round-half-to-even is done in fp32 via the magic constant 1.5*2^23.
 - ternary quantize sign(ws)*(|ws|>0.5) == RNE(clip(ws,-1,1)) exactly.
 - transposes to [k, r]/[k, n] layouts are identity matmuls (out = a.T @ I),
   batched 4 chunks per PSUM bank with one wide copy back to SBUF.
"""

import os

import numpy as np

import concourse.bass as bass
import concourse.mybir as mybir
import concourse.tile as tile
from concourse import bacc
from concourse.bass_utils import run_bass_kernel_spmd
from concourse.masks import make_identity
from concourse import bass_isa

F32 = mybir.dt.float32
BF16 = mybir.dt.bfloat16
ALU = mybir.AluOpType
AF = mybir.ActivationFunctionType

N_CORES = 8
R_FULL, K, N = 16384, 1024, 4096
R = R_FULL // N_CORES          # 2048 rows per core
RT = R // 128                  # 16 row tiles per core
KC = K // 128                  # 8 k-chunks
WS = N // 128                  # 32 weight strips (of 128 out-features)
WSH = WS // N_CORES            # 4 strips per core in the w_scale shard
NH = 2                         # n halves (2048 each)
NQ = 4                         # 512-wide psum tiles per half

C_MAGIC = 12582912.0           # 1.5 * 2^23: fp32 round-to-nearest-even trick
Q_EPS = 1e-5
NORM_EPS = 1e-6


def build_nc(g_is_ones: bool, use_collective: bool):
    nc = bacc.Bacc("TRN2", target_bir_lowering=False)

    x_d = nc.dram_tensor("x", [R, K], F32, kind="ExternalInput")
    w_d = nc.dram_tensor("w", [N, K], F32, kind="ExternalInput")
    if use_collective:
        wsh_d = nc.dram_tensor("wshard", [WSH * 128, K], F32, kind="ExternalInput")
        cc_in = nc.dram_tensor("cc_in", [1, 1], F32)
        cc_out = nc.dram_tensor("cc_out", [1, 1], F32, addr_space="Shared")
    if not g_is_ones:
        g_d = nc.dram_tensor("g", [1, K], F32, kind="ExternalInput")
    out_d = nc.dram_tensor("out", [R, N], F32, kind="ExternalOutput")

    with tile.TileContext(nc) as tc:
        with (
            tc.tile_pool(name="persist", bufs=1) as persist,
            tc.tile_pool(name="wst", bufs=6) as wst_pool,
            tc.tile_pool(name="wshp", bufs=2) as wsh_pool,
            tc.tile_pool(name="wscr", bufs=2) as wscr_pool,
            tc.tile_pool(name="xp", bufs=2) as x_pool,
            tc.tile_pool(name="xqTp", bufs=16) as xqT_pool,
            tc.tile_pool(name="csp", bufs=16) as cs_pool,
            tc.tile_pool(name="big", bufs=2) as big_pool,
            tc.tile_pool(name="stats", bufs=4) as st_pool,
            tc.tile_pool(name="osb", bufs=3) as osb_pool,
            tc.tile_pool(name="pmm", bufs=6, space="PSUM") as psum_mm,
            tc.tile_pool(name="ptp", bufs=2, space="PSUM") as psum_tp,
        ):
            # ---- constants ----
            ident = persist.tile([128, 128], BF16, tag="ident")
            make_identity(nc, ident[:])
            cb = persist.tile([128, 1], F32, tag="cb")
            nc.vector.memset(cb[:], C_MAGIC)

            if not g_is_ones:
                g_row = persist.tile([1, K], F32, tag="g_row")
                nc.sync.dma_start(g_row[:], g_d[:])
                g_b = persist.tile([128, K], F32, tag="g_b")
                nc.gpsimd.partition_broadcast(g_b[:], g_row[0:1, :])

            # w_t^T, bf16, split by n-half so matmuls only depend on the
            # strips of their half: wTT[h][:, j, n] = w_t[h*2048+n, j*128+kk]
            wTT = [
                persist.tile([128, KC, N // NH], BF16, tag=f"wTT{h}", name=f"wTT{h}")
                for h in range(NH)
            ]

            # ---- w_scale: sum of |w| ----
            nsum = WSH if use_collective else WS
            wpart = persist.tile([128, nsum], F32, tag="wpart")
            wsrc = wsh_d if use_collective else w_d
            with nc.named_scope("w_abs_sum"):
                for s in range(nsum):
                    wsha = wsh_pool.tile([128, K], F32, tag="wsha", name=f"wsha{s}")
                    nc.sync.dma_start(wsha[:], wsrc[s * 128:(s + 1) * 128, :])
                    wabs = wscr_pool.tile([128, K], BF16, tag="wabs", name=f"wabs{s}")
                    nc.scalar.activation(
                        wabs[:], wsha[:], AF.Abs,
                        accum_out=wpart[:, s:s + 1],
                    )
                wall = st_pool.tile([128, nsum], F32, tag="wall")
                nc.gpsimd.partition_all_reduce(
                    wall[:], wpart[:], channels=128,
                    reduce_op=bass_isa.ReduceOp.add)
                wsumb = st_pool.tile([128, 1], F32, tag="wsumb")
                nc.vector.reduce_sum(wsumb[:], wall[:], axis=mybir.AxisListType.X)

                if use_collective:
                    nc.sync.dma_start(cc_in[:], wsumb[0:1, :])
                    nc.gpsimd.collective_compute(
                        "AllReduce",
                        ALU.add,
                        replica_groups=[list(range(N_CORES))],
                        ins=[cc_in[:]],
                        outs=[cc_out[:]],
                    )
                    wsum_g1 = st_pool.tile([1, 1], F32, tag="wsum_g1")
                    nc.sync.dma_start(wsum_g1[:], cc_out[:])
                    wsum_g = st_pool.tile([128, 1], F32, tag="wsum_g")
                    nc.gpsimd.partition_broadcast(wsum_g[:], wsum_g1[0:1, :])
                else:
                    wsum_g = wsumb

                # w_scale = mean|w|; inv = 1/(w_scale + eps)  (all [128,1])
                wsb = persist.tile([128, 1], F32, tag="wsb")
                nc.vector.tensor_scalar(
                    out=wsb[:], in0=wsum_g[:], scalar1=1.0 / (N * K),
                    scalar2=None, op0=ALU.mult)
                speps1 = st_pool.tile([128, 1], F32, tag="speps1")
                nc.vector.tensor_scalar(
                    out=speps1[:], in0=wsum_g[:], scalar1=1.0 / (N * K),
                    scalar2=Q_EPS, op0=ALU.mult, op1=ALU.add)
                invb = persist.tile([128, 1], F32, tag="invb")
                nc.vector.reciprocal(invb[:], speps1[:])

            # ---- x quantization (independent of w) ----
            xqT_tiles = []
            cs_tiles = []

            def emit_x(rt):
                with nc.named_scope("x_quant"):
                    xt = x_pool.tile([128, K], F32, tag="xt", name=f"xt{rt}")
                    nc.scalar.dma_start(xt[:], x_d[rt * 128:(rt + 1) * 128, :])

                    if g_is_ones:
                        xg = xt
                    else:
                        xg = big_pool.tile([128, K], F32, tag="xg", name=f"xg{rt}")
                        nc.vector.tensor_mul(xg[:], xt[:], g_b[:])

                    xsq = big_pool.tile([128, K], BF16, tag="xsq", name=f"xsq{rt}")
                    ssq = st_pool.tile([128, 1], F32, tag="ssq")
                    nc.vector.scalar_tensor_tensor(
                        out=xsq[:], in0=xt[:], scalar=1.0, in1=xt[:],
                        op0=ALU.mult, op1=ALU.mult, accum_out=ssq[:])
                    am = st_pool.tile([128, 1], F32, tag="am")
                    nc.vector.tensor_reduce(
                        am[:], xg[:], axis=mybir.AxisListType.X, op=ALU.max,
                        apply_absolute_value=True)

                    # rs = 1/sqrt(ms + eps) with one Newton step on sqrt
                    ms = st_pool.tile([128, 1], F32, tag="ms")
                    nc.vector.tensor_scalar(
                        out=ms[:], in0=ssq[:], scalar1=1.0 / K,
                        scalar2=NORM_EPS, op0=ALU.mult, op1=ALU.add)
                    s0 = st_pool.tile([128, 1], F32, tag="s0")
                    nc.scalar.sqrt(s0[:], ms[:])
                    r0 = st_pool.tile([128, 1], F32, tag="r0")
                    nc.vector.reciprocal(r0[:], s0[:])
                    t0 = st_pool.tile([128, 1], F32, tag="t0")
                    nc.vector.tensor_mul(t0[:], ms[:], r0[:])
                    t1 = st_pool.tile([128, 1], F32, tag="t1")
                    nc.vector.tensor_add(t1[:], t0[:], s0[:])
                    s1 = st_pool.tile([128, 1], F32, tag="s1")
                    nc.vector.tensor_scalar(
                        out=s1[:], in0=t1[:], scalar1=0.5,
                        scalar2=None, op0=ALU.mult)
                    rs = st_pool.tile([128, 1], F32, tag="rs")
                    nc.vector.reciprocal(rs[:], s1[:])

                    axr = st_pool.tile([128, 1], F32, tag="axr")
                    nc.vector.tensor_mul(axr[:], am[:], rs[:])
                    xsc = st_pool.tile([128, 1], F32, tag="xsc")
                    nc.vector.tensor_scalar(
                        out=xsc[:], in0=axr[:], scalar1=1.0 / 127.0,
                        scalar2=None, op0=ALU.mult)
                    cs = cs_pool.tile([128, 1], F32, tag="cs", name=f"cs{rt}")
                    nc.vector.tensor_mul(cs[:], xsc[:], wsb[:])
                    sx = st_pool.tile([128, 1], F32, tag="sx")
                    nc.vector.tensor_scalar(
                        out=sx[:], in0=axr[:], scalar1=1.0 / 127.0,
                        scalar2=Q_EPS, op0=ALU.mult, op1=ALU.add)
                    dx = st_pool.tile([128, 1], F32, tag="dx")
                    nc.vector.reciprocal(dx[:], sx[:])
                    srow = st_pool.tile([128, 1], F32, tag="srow")
                    nc.vector.tensor_mul(srow[:], rs[:], dx[:])

                    # x_q = RNE(xg * srow) via +C (ACT) then -C (DVE, to bf16)
                    ux = big_pool.tile([128, K], F32, tag="ux", name=f"ux{rt}")
                    nc.scalar.activation(
                        ux[:], xg[:], AF.Identity,
                        bias=cb[:, 0:1], scale=srow[:, 0:1])
                    xq = big_pool.tile([128, K], BF16, tag="xq", name=f"xq{rt}")
                    nc.vector.tensor_scalar(
                        out=xq[:], in0=ux[:], scalar1=C_MAGIC,
                        scalar2=None, op0=ALU.subtract)

                    # transpose via identity matmuls, 4 chunks per psum
                    # bank, one wide copy back per bank
                    xqT = xqT_pool.tile([128, KC, 128], BF16, tag="xqT",
                                      name=f"xqT{rt}")
                    for g in range(KC // 4):
                        tpx = psum_tp.tile([128, 512], F32, tag="tp",
                                           name=f"tpx{rt}_{g}")
                        for jj in range(4):
                            j = g * 4 + jj
                            nc.tensor.matmul(
                                tpx[:, jj * 128:(jj + 1) * 128],
                                lhsT=xq[:, j * 128:(j + 1) * 128],
                                rhs=ident[:])
                        if g == 0:
                            nc.vector.tensor_copy(
                                xqT[:, g * 4:(g + 1) * 4, :], tpx[:])
                        else:
                            nc.scalar.copy(
                                xqT[:, g * 4:(g + 1) * 4, :], tpx[:])
                xqT_tiles.append(xqT)
                cs_tiles.append(cs)

            # ---- ternarize + transpose w strip (after thr; single pass when
            #      collective, re-streamed otherwise) ----
            def emit_w2(s):
                with nc.named_scope("w_ternarize"):
                    wst2 = wst_pool.tile([128, K], F32, tag="wst2", name=f"wst2{s}")
                    nc.sync.dma_start(wst2[:], w_d[s * 128:(s + 1) * 128, :])
                    u = wscr_pool.tile([128, K], F32, tag="wu", name=f"wu{s}")
                    nc.vector.tensor_scalar(
                        out=u[:], in0=wst2[:], scalar1=invb[:, 0:1],
                        scalar2=1.0, op0=ALU.mult, op1=ALU.min)
                    v = wscr_pool.tile([128, K], F32, tag="wv", name=f"wv{s}")
                    nc.vector.tensor_scalar(
                        out=v[:], in0=u[:], scalar1=-1.0,
                        scalar2=C_MAGIC, op0=ALU.max, op1=ALU.add)
                    wtn = wscr_pool.tile([128, K], BF16, tag="wtn", name=f"wtn{s}")
                    nc.scalar.activation(wtn[:], v[:], AF.Copy, bias=-C_MAGIC)

                    h, hcol = s // (WS // NH), (s % (WS // NH)) * 128
                    for g in range(KC // 4):
                        tpw = psum_tp.tile([128, 512], F32, tag="tp",
                                           name=f"tpw{s}_{g}")
                        for jj in range(4):
                            j = g * 4 + jj
                            nc.tensor.matmul(
                                tpw[:, jj * 128:(jj + 1) * 128],
                                lhsT=wtn[:, j * 128:(j + 1) * 128],
                                rhs=ident[:])
                        dst = wTT[h][:, g * 4:(g + 1) * 4, hcol:hcol + 128]
                        if g == 0:
                            nc.vector.tensor_copy(dst, tpw[:])
                        else:
                            nc.scalar.copy(dst, tpw[:])

            # ---- matmul + rescale for one (row-tile, n-half) ----
            def emit_mm(rt, h):
                xqT = xqT_tiles[rt]
                cs = cs_tiles[rt]
                with nc.named_scope("mm"):
                    pst = [
                        psum_mm.tile([128, 512], F32, tag="pmm",
                                     name=f"pmm_{rt}_{h}_{q}")
                        for q in range(NQ)
                    ]
                    for j in range(KC):
                        for q in range(NQ):
                            nc.tensor.matmul(
                                pst[q][:],
                                lhsT=xqT[:, j, :],
                                rhs=wTT[h][:, j, q * 512:(q + 1) * 512],
                                start=(j == 0), stop=(j == KC - 1))
                with nc.named_scope("out_scale"):
                    osbh = osb_pool.tile([128, N // NH], F32, tag="osb",
                                         name=f"osb{rt}_{h}")
                    for q in range(NQ):
                        dst = osbh[:, q * 512:(q + 1) * 512]
                        if q < NQ // 2:
                            nc.scalar.activation(
                                dst, pst[q][:], AF.Copy, scale=cs[:, 0:1])
                        else:
                            nc.vector.tensor_scalar(
                                out=dst, in0=pst[q][:], scalar1=cs[:, 0:1],
                                scalar2=None, op0=ALU.mult)
                    eng = nc.sync if (rt + h) % 2 == 0 else nc.scalar
                    eng.dma_start(
                        out_d[rt * 128:(rt + 1) * 128,
                              h * 2048:(h + 1) * 2048],
                        osbh[:])

            # ---- emission schedule ----
            # interleave x tiles with w strips; h0 matmuls join once their
            # dependencies are emitted; then drain h1.
            for i in range(8):
                emit_x(i)
                emit_w2(2 * i)
                emit_w2(2 * i + 1)
            for i in range(8, 16):
                emit_x(i)
                emit_w2(2 * i)
                emit_w2(2 * i + 1)
                emit_mm(i - 8, 0)
            for rt in range(8, 16):
                emit_mm(rt, 0)
                emit_mm(rt - 8, 1)
            for rt in range(8, 16):
                emit_mm(rt, 1)

    nc.compile()
    return nc


def _ensure_ntff_hook():
    """Make trace=True work: bass_utils imports antenv.axon_hooks, which is
    not present in this image. Shim it and install the ctypes-based NTFF
    profiling hook against libaxon_pjrt.so (same recipe as trn_boot)."""
    import sys
    import types
    try:
        import antenv.axon_hooks  # noqa: F401
        return
    except ImportError:
        pass
    mod = types.ModuleType("antenv.axon_hooks")
    mod._hook = None
    mod.set_axon_ntff_profile_hook = lambda h: setattr(mod, "_hook", h)
    mod.get_axon_ntff_profile_hook = lambda: mod._hook
    sys.modules["antenv.axon_hooks"] = mod
    import antenv
    antenv.axon_hooks = mod
    try:
        from trn_agent_boot.trn_boot import _ntff_profile_via_ctypes
        hook = _ntff_profile_via_ctypes("/opt/axon/libaxon_pjrt.so")
        if hook is not None:
            mod._hook = hook
    except Exception as e:  # degrade to no-trace
        print(f"ntff hook install failed: {e}")
    # no S3 in this sandbox; keep artifacts local
    import concourse.bass_utils as bu
    bu.upload_artifacts = lambda tmpdir: f"local://{tmpdir}"


_NC_CACHE = {}


def kernel(x: np.ndarray, weight: np.ndarray, norm_weight: np.ndarray) -> np.ndarray:
    x = np.ascontiguousarray(x, dtype=np.float32)
    weight = np.ascontiguousarray(weight, dtype=np.float32)
    norm_weight = np.ascontiguousarray(norm_weight, dtype=np.float32)

    B, S, Kin = x.shape
    xf = x.reshape(-1, Kin)
    g_is_ones = bool(np.all(norm_weight == 1.0))
    use_collective = bool(int(os.environ.get("BITLIN_COLLECTIVE", "1")))

    key = (g_is_ones, use_collective)
    if key not in _NC_CACHE:
        _NC_CACHE[key] = build_nc(g_is_ones, use_collective)
    nc = _NC_CACHE[key]

    in_maps = []
    for i in range(N_CORES):
        m = {"x": xf[i * R:(i + 1) * R], "w": weight}
        if use_collective:
            m["wshard"] = weight[i * WSH * 128:(i + 1) * WSH * 128]
        if not g_is_ones:
            m["g"] = norm_weight.reshape(1, Kin)
        in_maps.append(m)

    trace = bool(int(os.environ.get("BITLIN_TRACE", "0")))
    if trace:
        _ensure_ntff_hook()
    res = run_bass_kernel_spmd(
        nc, in_maps, core_ids=list(range(N_CORES)), trace=trace,
    )
    if trace:
        kernel.last_results = res
    out = np.concatenate([r["out"] for r in res.results], axis=0)
    return out.reshape(B, S, weight.shape[0]).astype(np.float32)

